# revision 8
# baseline (speedup 1.0000x reference)
"""Trainium2 Bass kernel for nn_AgentLayerC_v2 (segment_reduce).

Data-parallel over the B=64 segment dim: 8 contiguous segments (32768
points) per NeuronCore. All segment reductions are core-local; the small
GRU/MLP weights are replicated. The obs shard (16.8 MB) is kept
SBUF-resident so HBM reads it exactly once; both heavy reductions
(segment mean, priority-weighted segment sum) run as TensorE
column-reduce matmuls against the resident copy.
"""

import sys

if "/opt/trn_rl_repo" not in sys.path:
    sys.path.insert(0, "/opt/trn_rl_repo")

import numpy as np

import concourse.bass as bass
import concourse.bacc as bacc
import concourse.tile as tile
from concourse import mybir
from concourse.bass_utils import run_bass_kernel_spmd

F32 = mybir.dt.float32
F16 = mybir.dt.float16
AF = mybir.ActivationFunctionType

B = 64
PTS = 4096
N = B * PTS
OBS = 128
ACTD = 5
HID = 256
LAT = 64
CTX = 128
LOG2PIE = float(np.log(2.0 * np.pi * np.e))

NCORES = 8
SEG = B // NCORES          # 8 segments per core
PPC = SEG * PTS            # 32768 points per core
NT = PPC // 128            # 256 point-tiles of 128 per core
TPS = PTS // 128           # 32 tiles per segment
FREE = PPC // 128          # 256 free elems/partition in coh layout
XIN = LAT + ACTD + 1       # 70


def _build_bass():
    nc = bacc.Bacc("TRN2", target_bir_lowering=False, debug=False,
                   num_devices=NCORES)

    di = {}

    def inp(name, shape):
        di[name] = nc.dram_tensor(name, shape, F32, kind="ExternalInput")
        return di[name]

    def outp(name, shape):
        di[name] = nc.dram_tensor(name, shape, F32, kind="ExternalOutput")
        return di[name]

    # host pre-swizzles obs to partition-major: obs_pre[p, t*OBS+c] =
    # obs[t*128+p, c] -> each partition reads one contiguous run per chunk
    obs_t = di.setdefault("obs", nc.dram_tensor("obs", (128, NT * OBS), F16,
                                                 kind="ExternalInput"))
    coh_t = inp("coh", (PPC,))
    xT_t = inp("xT", (XIN, SEG))
    hpT_t = inp("hpT", (HID, SEG))
    wih_t = inp("wihT", (XIN, 3 * HID))
    whh_t = di.setdefault("whhT", nc.dram_tensor("whhT", (HID, 3 * HID), F16,
                                                 kind="ExternalInput"))
    hpT16_t = di.setdefault("hpT16", nc.dram_tensor("hpT16", (HID, SEG), F16,
                                                    kind="ExternalInput"))
    wpost_t = inp("wpostT", (HID + OBS, 2 * LAT))
    wae1_t = di.setdefault("wae1T", nc.dram_tensor("wae1T", (OBS, 256), F16,
                                                    kind="ExternalInput"))
    wae2_t = di.setdefault("wae2T", nc.dram_tensor("wae2T", (256, OBS), F16,
                                                   kind="ExternalInput"))
    wc1_t = di.setdefault("wc1T", nc.dram_tensor("wc1T", (HID + LAT + OBS, 512),
                                                 F16, kind="ExternalInput"))
    wc2_t = di.setdefault("wc2T", nc.dram_tensor("wc2T", (512, CTX), F16,
                                                 kind="ExternalInput"))
    bsum_t = inp("bsumT", (128, 4))
    bihn_t = inp("bihnT", (128, 2))
    bhhn_t = inp("bhhnT", (128, 2))
    bpmu_t = inp("bpostMu", (LAT, 1))
    bplv_t = inp("bpostLv", (LAT, 1))
    bae1_t = inp("bae1T", (128, 2))
    bae2_t = inp("bae2T", (128, 1))
    bc1_t = inp("bc1T", (128, 4))
    bc2_t = inp("bc2T", (128, 1))
    ident_t = inp("ident", (128, 128))
    rmat_t = inp("rmat", (SEG, 128))
    ggmat_t = inp("ggmat", (128, 128))

    prio_t = outp("prio", (PPC,))
    pn_t = outp("pn", (PPC,))
    hT_t = outp("hT", (HID, SEG))
    zT_t = outp("zT", (LAT, SEG))
    ctxT_t = outp("ctxT", (CTX, SEG))
    unc_t = outp("unc", (SEG, 1))

    with tile.TileContext(nc) as tc:
        with (
            tc.tile_pool(name="consts", bufs=1) as consts,
            tc.tile_pool(name="obsp", bufs=1) as obsp,
            tc.tile_pool(name="work", bufs=1) as work,
            tc.tile_pool(name="ps", bufs=3, space="PSUM") as ps,
            tc.tile_pool(name="psacc", bufs=1, space="PSUM") as psacc,
            tc.tile_pool(name="psg", bufs=1, space="PSUM") as psg,
            tc.tile_pool(name="psmlp", bufs=2, space="PSUM") as psmlp,
        ):
            # ---- obs resident load first: it owns the HWDGE queues ----
            obs_sb = obsp.tile([128, NT, OBS], F16, tag="obs")
            NCHUNK = 16
            CW = NT * OBS // NCHUNK
            TPC = NT // NCHUNK
            for g in range(NCHUNK):
                nc.sync.dma_start(out=obs_sb[:, g * TPC:(g + 1) * TPC, :],
                                  in_=obs_t.ap()[:, g * CW:(g + 1) * CW])

            # ---- constants / weights into SBUF (gpsimd queue) ----
            def load(name, shape, src_ap, dt=F32):
                t = consts.tile(shape, dt, tag=name)
                nc.gpsimd.dma_start(out=t[:], in_=src_ap)
                return t

            ident = load("ident", [128, 128], ident_t.ap())
            rmat = load("rmat", [SEG, 128], rmat_t.ap())
            ggmat = load("ggmat", [128, 128], ggmat_t.ap())
            xT = load("xT", [XIN, SEG], xT_t.ap())
            hpT = load("hpT", [128, 2, SEG],
                       hpT_t.ap().rearrange("(c p) s -> p c s", p=128))
            wih = load("wih", [XIN, 3 * HID], wih_t.ap())
            whh = load("whh", [128, 2, 3 * HID],
                       whh_t.ap().rearrange("(c p) n -> p c n", p=128), dt=F16)
            hpT16 = load("hpT16", [128, 2, SEG],
                         hpT16_t.ap().rearrange("(c p) s -> p c s", p=128),
                         dt=F16)
            wpost = load("wpost", [128, 3, 2 * LAT],
                         wpost_t.ap().rearrange("(c p) n -> p c n", p=128))
            wae1 = load("wae1", [OBS, 256], wae1_t.ap(), dt=F16)
            wae2 = load("wae2", [128, 2, OBS],
                        wae2_t.ap().rearrange("(c p) n -> p c n", p=128), dt=F16)
            wc1_c0 = load("wc1_c0", [128, 512], wc1_t.ap()[0:128, :], dt=F16)
            wc1_c1 = load("wc1_c1", [128, 512], wc1_t.ap()[128:256, :], dt=F16)
            wc1_c2 = load("wc1_c2", [LAT, 512], wc1_t.ap()[256:320, :], dt=F16)
            wc1_c3 = load("wc1_c3", [128, 512], wc1_t.ap()[320:448, :], dt=F16)
            wc2 = load("wc2", [128, 4, CTX],
                       wc2_t.ap().rearrange("(c p) n -> p c n", p=128), dt=F16)
            bsum = load("bsum", [128, 4], bsum_t.ap())
            bihn = load("bihn", [128, 2], bihn_t.ap())
            bhhn = load("bhhn", [128, 2], bhhn_t.ap())
            bpmu = load("bpmu", [LAT, 1], bpmu_t.ap())
            bplv = load("bplv", [LAT, 1], bplv_t.ap())
            bae1 = load("bae1", [128, 2], bae1_t.ap())
            bae2 = load("bae2", [128, 1], bae2_t.ap())
            bc1 = load("bc1", [128, 4], bc1_t.ap())
            bc2 = load("bc2", [128, 1], bc2_t.ap())

            onespad = consts.tile([128, 2 * SEG - 1, 1], F16, tag="onespad")
            nc.vector.memset(onespad[:], 0.0)
            nc.vector.memset(onespad[:, SEG - 1, :], 1.0)
            ones64 = consts.tile([LAT, 1], F32, tag="ones64")
            nc.vector.memset(ones64[:], 1.0)
            neg1 = consts.tile([1, 1], F32, tag="neg1")
            nc.vector.memset(neg1[:], -1.0)

            coh = work.tile([128, FREE], F32, tag="coh")
            nc.gpsimd.dma_start(out=coh[:],
                                in_=coh_t.ap().rearrange("(p f) -> p f", p=128))

            # ---- phase 1: per-segment column sums of obs ----
            # lhsT = 8-wide window into a zero-padded ones column; window
            # offset puts the 1.0 column at position s, zeros elsewhere, so
            # every tile accumulates into row s of one (8, 128) PSUM tile.
            sum8_ps = psacc.tile([SEG, OBS], F32, tag="acc")
            for t in range(NT):
                s = t // TPS
                nc.tensor.matmul(sum8_ps[:], rhs=obs_sb[:, t, :],
                                 lhsT=onespad[:, SEG - 1 - s:2 * SEG - 1 - s, :],
                                 start=(t == 0), stop=(t == NT - 1))
            sum8 = work.tile([SEG, OBS], F32, tag="sum8")
            nc.vector.tensor_copy(sum8[:], sum8_ps[:])
            aggT_ps = ps.tile([128, SEG], F32, tag="sm")
            nc.tensor.matmul(aggT_ps[:], lhsT=sum8[:], rhs=ident[0:SEG, 0:SEG],
                             start=True, stop=True)
            aggT = work.tile([128, SEG], F32, tag="aggT")
            nc.vector.tensor_scalar_mul(aggT[:], aggT_ps[:], 1.0 / PTS)

            # ---- GRU over [z_prev, action, coh_scalar] (transposed) ----
            gall_ps = psg.tile([128, 8, SEG], F32, tag="gall")
            g_ps = gall_ps[:, 0:4, :]
            gin_ps = gall_ps[:, 4:6, :]
            ghn_ps = gall_ps[:, 6:8, :]
            for j in range(4):
                nc.tensor.matmul(g_ps[:, j, :],
                                 lhsT=wih[:, j * 128:(j + 1) * 128], rhs=xT[:],
                                 start=True, stop=False)
                for c in range(2):
                    nc.tensor.matmul(g_ps[:, j, :],
                                     lhsT=whh[:, c, j * 128:(j + 1) * 128],
                                     rhs=hpT16[:, c, :], start=False,
                                     stop=(c == 1))
            for j in range(2):
                jj = 4 + j
                nc.tensor.matmul(gin_ps[:, j, :],
                                 lhsT=wih[:, jj * 128:(jj + 1) * 128],
                                 rhs=xT[:], start=True, stop=True)
                for c in range(2):
                    nc.tensor.matmul(ghn_ps[:, j, :],
                                     lhsT=whh[:, c, jj * 128:(jj + 1) * 128],
                                     rhs=hpT16[:, c, :], start=(c == 0),
                                     stop=(c == 1))

            r_sb = work.tile([128, 2, SEG], F32, tag="r")
            u_sb = work.tile([128, 2, SEG], F32, tag="u")
            for j in range(2):
                nc.scalar.activation(r_sb[:, j, :], g_ps[:, j, :], AF.Sigmoid,
                                     bias=bsum[:, j:j + 1])
                nc.scalar.activation(u_sb[:, j, :], g_ps[:, 2 + j, :],
                                     AF.Sigmoid, bias=bsum[:, 2 + j:3 + j])
            hT = work.tile([128, 2, SEG], F32, tag="hT")
            t1 = work.tile([128, 2, SEG], F32, tag="t1")
            t2 = work.tile([128, 2, SEG], F32, tag="t2")
            t3 = work.tile([128, 2, SEG], F32, tag="t3")
            n_sb = work.tile([128, 2, SEG], F32, tag="n")
            omu = work.tile([128, 2, SEG], F32, tag="omu")
            t5 = work.tile([128, 2, SEG], F32, tag="t5")
            t6 = work.tile([128, 2, SEG], F32, tag="t6")
            for j in range(2):
                # n = tanh(gi_n + b_ihn + r * (gh_n + b_hhn))
                nc.scalar.activation(t1[:, j, :], ghn_ps[:, j, :], AF.Identity,
                                     bias=bhhn[:, j:j + 1])
                nc.vector.tensor_mul(t2[:, j, :], r_sb[:, j, :], t1[:, j, :])
                nc.vector.tensor_add(t3[:, j, :], t2[:, j, :], gin_ps[:, j, :])
                nc.scalar.activation(n_sb[:, j, :], t3[:, j, :], AF.Tanh,
                                     bias=bihn[:, j:j + 1])
                # h = (1 - u) * n + u * h_prev
                nc.vector.tensor_scalar(omu[:, j, :], u_sb[:, j, :], -1.0, 1.0,
                                        op0=mybir.AluOpType.mult,
                                        op1=mybir.AluOpType.add)
                nc.vector.tensor_mul(t5[:, j, :], omu[:, j, :], n_sb[:, j, :])
                nc.vector.tensor_mul(t6[:, j, :], u_sb[:, j, :], hpT[:, j, :])
                nc.vector.tensor_add(hT[:, j, :], t5[:, j, :], t6[:, j, :])
            nc.sync.dma_start(out=hT_t.ap().rearrange("(c p) s -> p c s", p=128),
                              in_=hT[:])

            # ---- posterior head: mu_q (=z), logvar_q, uncertainty ----
            muq_ps = ps.tile([LAT, SEG], F32, tag="sm")
            lvq_ps = ps.tile([LAT, SEG], F32, tag="sm")
            post_rhs = [hT[:, 0, :], hT[:, 1, :], aggT[:]]
            for c in range(3):
                nc.tensor.matmul(muq_ps[:], lhsT=wpost[:, c, 0:LAT],
                                 rhs=post_rhs[c], start=(c == 0), stop=(c == 2))
                nc.tensor.matmul(lvq_ps[:], lhsT=wpost[:, c, LAT:2 * LAT],
                                 rhs=post_rhs[c], start=(c == 0), stop=(c == 2))
            zT = work.tile([LAT, SEG], F32, tag="zT")
            lvqT = work.tile([LAT, SEG], F32, tag="lvqT")
            nc.scalar.activation(zT[:], muq_ps[:], AF.Identity, bias=bpmu[:])
            nc.scalar.activation(lvqT[:], lvq_ps[:], AF.Identity, bias=bplv[:])
            nc.sync.dma_start(out=zT_t.ap(), in_=zT[:])

            uncs_ps = ps.tile([SEG, 1], F32, tag="sm")
            nc.tensor.matmul(uncs_ps[:], lhsT=lvqT[:], rhs=ones64[:],
                             start=True, stop=True)
            unc_bias = consts.tile([SEG, 1], F32, tag="unc_bias")
            nc.vector.memset(unc_bias[:], 0.5 * LAT * LOG2PIE)
            unc_sb = work.tile([SEG, 1], F32, tag="unc")
            nc.scalar.activation(unc_sb[:], uncs_ps[:], AF.Identity,
                                 bias=unc_bias[:], scale=0.5)
            nc.sync.dma_start(out=unc_t.ap(), in_=unc_sb[:])

            # ---- per-point priority + segment softmax ----
            # coh layout: partition p holds points [p*256, (p+1)*256);
            # segment of partition p is p // 16.
            unc128_ps = ps.tile([128, 1], F32, tag="sm")
            nc.tensor.matmul(unc128_ps[:], lhsT=rmat[:], rhs=unc_sb[:],
                             start=True, stop=True)
            unc128 = work.tile([128, 1], F32, tag="unc128")
            nc.vector.tensor_copy(unc128[:], unc128_ps[:])

            prio = work.tile([128, FREE], F32, tag="prio")
            nc.vector.tensor_scalar_mul(prio[:], coh[:], unc128[:])
            nc.sync.dma_start(out=prio_t.ap().rearrange("(p f) -> p f", p=128),
                              in_=prio[:])

            pmax = work.tile([128, 1], F32, tag="pmax")
            nc.vector.reduce_max(pmax[:], prio[:], axis=mybir.AxisListType.X)
            pmaxT_ps = ps.tile([1, 128], F32, tag="sm")
            nc.tensor.matmul(pmaxT_ps[:], lhsT=pmax[:], rhs=ident[:],
                             start=True, stop=True)
            mT = work.tile([1, SEG], F32, tag="mT")
            nc.vector.reduce_max(mT[:],
                                 pmaxT_ps[:].rearrange("p (a b) -> p a b", b=16),
                                 axis=mybir.AxisListType.X)
            # negm128[p] = -mT[0, p//16]: expand the repeat on DVE (matmul
            # weight APs allow only one free dim), then one K=1 matmul
            mT128 = work.tile([1, 128], F32, tag="mT128")
            nc.vector.tensor_copy(mT128[:].rearrange("p (a b) -> p a b", b=16),
                                  mT[:].to_broadcast((1, SEG, 16)))
            negm128_ps = ps.tile([128, 1], F32, tag="sm")
            nc.tensor.matmul(negm128_ps[:], lhsT=mT128[:],
                             rhs=neg1[:], start=True, stop=True)
            negm128 = work.tile([128, 1], F32, tag="negm128")
            nc.vector.tensor_copy(negm128[:], negm128_ps[:])

            e_sb = work.tile([128, FREE], F32, tag="e")
            nc.scalar.activation(e_sb[:], prio[:], AF.Exp, bias=negm128[:])
            esum = work.tile([128, 1], F32, tag="esum")
            nc.vector.reduce_sum(esum[:], e_sb[:], axis=mybir.AxisListType.X)
            # den128[p] = sum over p's 16-partition group = GG.T @ esum in one
            # matmul (GG is the block-diagonal ones matrix)
            den128_ps = ps.tile([128, 1], F32, tag="sm")
            nc.tensor.matmul(den128_ps[:], lhsT=ggmat[:], rhs=esum[:],
                             start=True, stop=True)
            rden128 = work.tile([128, 1], F32, tag="rden128")
            nc.vector.reciprocal(rden128[:], den128_ps[:])

            pn_sb = work.tile([128, FREE], F32, tag="pn")
            nc.vector.tensor_scalar_mul(pn_sb[:], e_sb[:], rden128[:])
            nc.sync.dma_start(out=pn_t.ap().rearrange("(p f) -> p f", p=128),
                              in_=pn_sb[:])

            # transpose pn into point-on-partition columns inside a
            # zero-padded zone buffer (zone SEG-1 holds the data):
            # col b*128+q of the zone = pn for points q*256 + b*128 + [0,128)
            pnpad = work.tile([128, 2 * OBS, 2 * SEG - 1], F16, tag="pnpad")
            nc.vector.memset(pnpad[:], 0.0)
            for b in range(2):
                pnT_ps = ps.tile([128, 128], F32, tag="sm")
                nc.tensor.matmul(pnT_ps[:], lhsT=pn_sb[:, b * 128:(b + 1) * 128],
                                 rhs=ident[:], start=True, stop=True)
                nc.vector.tensor_copy(pnpad[:, b * 128:(b + 1) * 128, SEG - 1],
                                      pnT_ps[:])

            # ---- phase C: attended = sum_i pn_i * obs_i per segment ----
            att8_ps = psacc.tile([SEG, OBS], F32, tag="acc")
            for t in range(NT):
                s = t // TPS
                cc = (t % 2) * 128 + t // 2
                nc.tensor.matmul(att8_ps[:], rhs=obs_sb[:, t, :],
                                 lhsT=pnpad[:, cc, SEG - 1 - s:2 * SEG - 1 - s],
                                 start=(t == 0), stop=(t == NT - 1))
            att8 = work.tile([SEG, OBS], F32, tag="att8")
            nc.vector.tensor_copy(att8[:], att8_ps[:])
            attT_ps2 = ps.tile([128, SEG], F32, tag="sm")
            nc.tensor.matmul(attT_ps2[:], lhsT=att8[:], rhs=ident[0:SEG, 0:SEG],
                             start=True, stop=True)
            attT = work.tile([128, SEG], F16, tag="attT")
            nc.vector.tensor_copy(attT[:], attT_ps2[:])
            hT16 = work.tile([128, 2, SEG], F16, tag="hT16")
            nc.vector.tensor_copy(hT16[:], hT[:])
            zT16 = work.tile([LAT, SEG], F16, tag="zT16")
            nc.vector.tensor_copy(zT16[:], zT[:])

            # ---- obs_enc MLP ----
            ae_ps = psmlp.tile([128, 2, SEG], F32, tag="mlp")
            for j in range(2):
                nc.tensor.matmul(ae_ps[:, j, :],
                                 lhsT=wae1[:, j * 128:(j + 1) * 128],
                                 rhs=attT[:], start=True, stop=True)
            t1ae = work.tile([128, 2, SEG], F16, tag="t1ae")
            for j in range(2):
                nc.scalar.activation(t1ae[:, j, :], ae_ps[:, j, :], AF.Relu,
                                     bias=bae1[:, j:j + 1])
            enc_ps = psmlp.tile([128, 1, SEG], F32, tag="mlp")
            for c in range(2):
                nc.tensor.matmul(enc_ps[:, 0, :], lhsT=wae2[:, c, :],
                                 rhs=t1ae[:, c, :], start=(c == 0),
                                 stop=(c == 1))
            encT = work.tile([128, SEG], F16, tag="encT")
            nc.scalar.activation(encT[:], enc_ps[:, 0, :], AF.Identity,
                                 bias=bae2[:])

            # ---- context MLP ----
            c1_ps = psmlp.tile([128, 4, SEG], F32, tag="mlp")
            c1_rhs = [(wc1_c0, hT16[:, 0, :]), (wc1_c1, hT16[:, 1, :]),
                      (wc1_c2, zT16[:]), (wc1_c3, encT[:])]
            for j in range(4):
                for c, (w, rhs) in enumerate(c1_rhs):
                    nc.tensor.matmul(c1_ps[:, j, :],
                                     lhsT=w[:, j * 128:(j + 1) * 128],
                                     rhs=rhs, start=(c == 0), stop=(c == 3))
            c1_sb = work.tile([128, 4, SEG], F16, tag="c1")
            for j in range(4):
                nc.scalar.activation(c1_sb[:, j, :], c1_ps[:, j, :], AF.Relu,
                                     bias=bc1[:, j:j + 1])
            ctx_ps = psmlp.tile([128, 1, SEG], F32, tag="mlp")
            for c in range(4):
                nc.tensor.matmul(ctx_ps[:, 0, :], lhsT=wc2[:, c, :],
                                 rhs=c1_sb[:, c, :], start=(c == 0),
                                 stop=(c == 3))
            ctxT = work.tile([CTX, SEG], F32, tag="ctxT")
            nc.scalar.activation(ctxT[:], ctx_ps[:, 0, :], AF.Identity,
                                 bias=bc2[:])
            nc.sync.dma_start(out=ctxT_t.ap(), in_=ctxT[:])

    nc.compile()
    return nc


_NC_CACHE = None


def _get_nc():
    global _NC_CACHE
    if _NC_CACHE is None:
        _NC_CACHE = _build_bass()
    return _NC_CACHE


def _host_consts(action, coh_scalar, h_prev, z_prev, W_ih, b_ih, W_hh, b_hh,
                 W_post, b_post, W_ae1, b_ae1, W_ae2, b_ae2, W_c1, b_c1,
                 W_c2, b_c2):
    c = lambda a: np.ascontiguousarray(a, dtype=np.float32)
    x = np.concatenate([z_prev, action, coh_scalar], axis=1)  # (B, 70)
    bsum = (b_ih + b_hh)[:512].reshape(4, 128).T
    rmat = np.zeros((SEG, 128), np.float32)
    rmat[np.arange(128) // 16, np.arange(128)] = 1.0
    consts = {
        "wihT": c(W_ih.T), "whhT": c(W_hh.T).astype(np.float16),
        "wpostT": c(W_post.T),
        "wae1T": c(W_ae1.T).astype(np.float16),
        "wae2T": c(W_ae2.T).astype(np.float16),
        "wc1T": c(W_c1.T).astype(np.float16),
        "wc2T": c(W_c2.T).astype(np.float16),
        "bsumT": c(bsum),
        "bihnT": c(b_ih[512:].reshape(2, 128).T),
        "bhhnT": c(b_hh[512:].reshape(2, 128).T),
        "bpostMu": c(b_post[:LAT, None]), "bpostLv": c(b_post[LAT:, None]),
        "bae1T": c(b_ae1.reshape(2, 128).T), "bae2T": c(b_ae2[:, None]),
        "bc1T": c(b_c1.reshape(4, 128).T), "bc2T": c(b_c2[:, None]),
        "ident": np.eye(128, dtype=np.float32),
        "rmat": rmat,
        "ggmat": (rmat.T @ rmat).astype(np.float32),
    }
    return x, consts


def _reference_numpy(obs, action, coh_scalar, coh_spatial, h_prev, z_prev,
                     batch, W_ih, b_ih, W_hh, b_hh, W_prior, b_prior, W_post,
                     b_post, W_ae1, b_ae1, W_ae2, b_ae2, W_c1, b_c1, W_c2,
                     b_c2):
    """Pure-numpy fallback for a batch layout the device path doesn't cover."""
    def seg_sum(x, idx, nseg):
        out = np.zeros((nseg,) + x.shape[1:], np.float32)
        np.add.at(out, idx, x)
        return out

    nb = batch.astype(np.int64)
    counts = seg_sum(np.ones(len(nb), np.float32), nb, B)
    obs_sum = seg_sum(obs, nb, B)
    obs_agg = obs_sum / np.maximum(counts, 1.0)[:, None]
    x = np.concatenate([z_prev, action, coh_scalar], axis=-1)
    gi = x @ W_ih.T + b_ih
    gh = h_prev @ W_hh.T + b_hh
    gi_r, gi_z, gi_n = np.split(gi, 3, axis=-1)
    gh_r, gh_z, gh_n = np.split(gh, 3, axis=-1)
    sig = lambda v: 1.0 / (1.0 + np.exp(-v))
    r = sig(gi_r + gh_r)
    u = sig(gi_z + gh_z)
    n = np.tanh(gi_n + r * gh_n)
    h = (1.0 - u) * n + u * h_prev
    post = np.concatenate([h, obs_agg], axis=-1) @ W_post.T + b_post
    mu_q, logvar_q = np.split(post, 2, axis=-1)
    z = mu_q
    unc = 0.5 * np.sum(logvar_q + LOG2PIE, axis=-1)
    priority = coh_spatial * unc[nb]
    s = priority
    m = np.full((B,), -np.inf, np.float32)
    np.maximum.at(m, nb, s)
    e = np.exp(s - m[nb])
    denom = seg_sum(e, nb, B)
    pn = e / np.maximum(denom, 1e-12)[nb]
    att = seg_sum(obs * pn[:, None], nb, B)
    enc = np.maximum(att @ W_ae1.T + b_ae1, 0.0) @ W_ae2.T + b_ae2
    ctx_in = np.concatenate([h, z, enc], axis=-1)
    context = np.maximum(ctx_in @ W_c1.T + b_c1, 0.0) @ W_c2.T + b_c2
    return (h.astype(np.float32), z.astype(np.float32),
            context.astype(np.float32), priority.astype(np.float32),
            pn.astype(np.float32), unc.astype(np.float32))


def kernel(**inputs):
    f = {k: np.asarray(v) for k, v in inputs.items()}
    batch = f["batch"]
    expected = np.repeat(np.arange(B, dtype=batch.dtype), PTS)
    if batch.shape != expected.shape or not np.array_equal(batch, expected):
        return _reference_numpy(**{k: (np.asarray(v, np.float32)
                                       if k != "batch" else v)
                                   for k, v in f.items()})

    g = {k: np.ascontiguousarray(np.asarray(v), dtype=np.float32)
         for k, v in f.items() if k != "batch"}
    x, consts = _host_consts(
        g["action"], g["coh_scalar"], g["h_prev"], g["z_prev"],
        g["W_ih"], g["b_ih"], g["W_hh"], g["b_hh"], g["W_post"], g["b_post"],
        g["W_ae1"], g["b_ae1"], g["W_ae2"], g["b_ae2"], g["W_c1"], g["b_c1"],
        g["W_c2"], g["b_c2"])

    obs16 = g["obs"].astype(np.float16)
    in_maps = []
    for d in range(NCORES):
        sl = slice(d * SEG, (d + 1) * SEG)
        psl = slice(d * PPC, (d + 1) * PPC)
        m = dict(consts)
        m["obs"] = np.ascontiguousarray(
            obs16[psl].reshape(NT, 128, OBS).transpose(1, 0, 2)
            .reshape(128, NT * OBS))
        m["coh"] = np.ascontiguousarray(g["coh_spatial"][psl])
        m["xT"] = np.ascontiguousarray(x[sl].T)
        m["hpT"] = np.ascontiguousarray(g["h_prev"][sl].T)
        m["hpT16"] = m["hpT"].astype(np.float16)
        in_maps.append(m)

    nc = _get_nc()
    res = run_bass_kernel_spmd(nc, in_maps, core_ids=list(range(NCORES)))

    h = np.empty((B, HID), np.float32)
    z = np.empty((B, LAT), np.float32)
    context = np.empty((B, CTX), np.float32)
    priority = np.empty((N,), np.float32)
    pn = np.empty((N,), np.float32)
    unc = np.empty((B,), np.float32)
    for d in range(NCORES):
        r = res.results[d]
        sl = slice(d * SEG, (d + 1) * SEG)
        psl = slice(d * PPC, (d + 1) * PPC)
        h[sl] = r["hT"].T
        z[sl] = r["zT"].T
        context[sl] = r["ctxT"].T
        priority[psl] = r["prio"]
        pn[psl] = r["pn"]
        unc[sl] = r["unc"][:, 0]
    return h, z, context, priority, pn, unc


# revision 9
# speedup vs baseline: 1.0174x; 1.0174x over previous
"""Trainium2 Bass kernel for nn_AgentLayerC_v2 (segment_reduce).

Data-parallel over the B=64 segment dim: 8 contiguous segments (32768
points) per NeuronCore. All segment reductions are core-local; the small
GRU/MLP weights are replicated. The obs shard (16.8 MB) is kept
SBUF-resident so HBM reads it exactly once; both heavy reductions
(segment mean, priority-weighted segment sum) run as TensorE
column-reduce matmuls against the resident copy.
"""

import sys

if "/opt/trn_rl_repo" not in sys.path:
    sys.path.insert(0, "/opt/trn_rl_repo")

import numpy as np

import concourse.bass as bass
import concourse.bacc as bacc
import concourse.tile as tile
from concourse import mybir
from concourse.bass_utils import run_bass_kernel_spmd

F32 = mybir.dt.float32
F16 = mybir.dt.float16
AF = mybir.ActivationFunctionType

B = 64
PTS = 4096
N = B * PTS
OBS = 128
ACTD = 5
HID = 256
LAT = 64
CTX = 128
LOG2PIE = float(np.log(2.0 * np.pi * np.e))

NCORES = 8
SEG = B // NCORES          # 8 segments per core
PPC = SEG * PTS            # 32768 points per core
NT = PPC // 128            # 256 point-tiles of 128 per core
TPS = PTS // 128           # 32 tiles per segment
FREE = PPC // 128          # 256 free elems/partition in coh layout
XIN = LAT + ACTD + 1       # 70


def _build_bass():
    nc = bacc.Bacc("TRN2", target_bir_lowering=False, debug=False,
                   num_devices=NCORES)

    di = {}

    def inp(name, shape):
        di[name] = nc.dram_tensor(name, shape, F32, kind="ExternalInput")
        return di[name]

    def outp(name, shape):
        di[name] = nc.dram_tensor(name, shape, F32, kind="ExternalOutput")
        return di[name]

    # host pre-swizzles obs to partition-major: obs_pre[p, t*OBS+c] =
    # obs[t*128+p, c] -> each partition reads one contiguous run per chunk
    obs_t = di.setdefault("obs", nc.dram_tensor("obs", (128, NT * OBS), F16,
                                                 kind="ExternalInput"))
    coh_t = inp("coh", (PPC,))
    xT_t = inp("xT", (XIN, SEG))
    hpT_t = inp("hpT", (HID, SEG))
    wih_t = inp("wihT", (XIN, 3 * HID))
    whh_t = di.setdefault("whhT", nc.dram_tensor("whhT", (HID, 3 * HID), F16,
                                                 kind="ExternalInput"))
    hpT16_t = di.setdefault("hpT16", nc.dram_tensor("hpT16", (HID, SEG), F16,
                                                    kind="ExternalInput"))
    wpost_t = inp("wpostT", (HID + OBS, 2 * LAT))
    wae1_t = di.setdefault("wae1T", nc.dram_tensor("wae1T", (OBS, 256), F16,
                                                    kind="ExternalInput"))
    wae2_t = di.setdefault("wae2T", nc.dram_tensor("wae2T", (256, OBS), F16,
                                                   kind="ExternalInput"))
    wc1_t = di.setdefault("wc1T", nc.dram_tensor("wc1T", (HID + LAT + OBS, 512),
                                                 F16, kind="ExternalInput"))
    wc2_t = di.setdefault("wc2T", nc.dram_tensor("wc2T", (512, CTX), F16,
                                                 kind="ExternalInput"))
    bsum_t = inp("bsumT", (128, 4))
    bihn_t = inp("bihnT", (128, 2))
    bhhn_t = inp("bhhnT", (128, 2))
    bpmu_t = inp("bpostMu", (LAT, 1))
    bplv_t = inp("bpostLv", (LAT, 1))
    bae1_t = inp("bae1T", (128, 2))
    bae2_t = inp("bae2T", (128, 1))
    bc1_t = inp("bc1T", (128, 4))
    bc2_t = inp("bc2T", (128, 1))
    ident_t = inp("ident", (128, 128))
    rmat_t = inp("rmat", (SEG, 128))
    ggmat_t = inp("ggmat", (128, 128))

    prio_t = outp("prio", (PPC,))
    pn_t = outp("pn", (PPC,))
    hT_t = outp("hT", (HID, SEG))
    zT_t = outp("zT", (LAT, SEG))
    ctxT_t = outp("ctxT", (CTX, SEG))
    unc_t = outp("unc", (SEG, 1))

    with tile.TileContext(nc) as tc:
        with (
            tc.tile_pool(name="consts", bufs=1) as consts,
            tc.tile_pool(name="obsp", bufs=1) as obsp,
            tc.tile_pool(name="work", bufs=1) as work,
            tc.tile_pool(name="ps", bufs=3, space="PSUM") as ps,
            tc.tile_pool(name="psacc", bufs=1, space="PSUM") as psacc,
            tc.tile_pool(name="psg", bufs=1, space="PSUM") as psg,
            tc.tile_pool(name="psmlp", bufs=2, space="PSUM") as psmlp,
        ):
            # ---- obs resident load first: it owns the HWDGE queues ----
            obs_sb = obsp.tile([128, NT, OBS], F16, tag="obs")
            NCHUNK = 16
            CW = NT * OBS // NCHUNK
            TPC = NT // NCHUNK
            for g in range(NCHUNK):
                nc.sync.dma_start(out=obs_sb[:, g * TPC:(g + 1) * TPC, :],
                                  in_=obs_t.ap()[:, g * CW:(g + 1) * CW])

            # ---- constants / weights into SBUF (gpsimd queue) ----
            def load(name, shape, src_ap, dt=F32):
                t = consts.tile(shape, dt, tag=name)
                nc.gpsimd.dma_start(out=t[:], in_=src_ap)
                return t

            ident = load("ident", [128, 128], ident_t.ap())
            rmat = load("rmat", [SEG, 128], rmat_t.ap())
            ggmat = load("ggmat", [128, 128], ggmat_t.ap())
            xT = load("xT", [XIN, SEG], xT_t.ap())
            hpT = load("hpT", [128, 2, SEG],
                       hpT_t.ap().rearrange("(c p) s -> p c s", p=128))
            wih = load("wih", [XIN, 3 * HID], wih_t.ap())
            whh = load("whh", [128, 2, 3 * HID],
                       whh_t.ap().rearrange("(c p) n -> p c n", p=128), dt=F16)
            hpT16 = load("hpT16", [128, 2, SEG],
                         hpT16_t.ap().rearrange("(c p) s -> p c s", p=128),
                         dt=F16)
            wpost = load("wpost", [128, 3, 2 * LAT],
                         wpost_t.ap().rearrange("(c p) n -> p c n", p=128))
            wae1 = load("wae1", [OBS, 256], wae1_t.ap(), dt=F16)
            wae2 = load("wae2", [128, 2, OBS],
                        wae2_t.ap().rearrange("(c p) n -> p c n", p=128), dt=F16)
            wc1_c0 = load("wc1_c0", [128, 512], wc1_t.ap()[0:128, :], dt=F16)
            wc1_c1 = load("wc1_c1", [128, 512], wc1_t.ap()[128:256, :], dt=F16)
            wc1_c2 = load("wc1_c2", [LAT, 512], wc1_t.ap()[256:320, :], dt=F16)
            wc1_c3 = load("wc1_c3", [128, 512], wc1_t.ap()[320:448, :], dt=F16)
            wc2 = load("wc2", [128, 4, CTX],
                       wc2_t.ap().rearrange("(c p) n -> p c n", p=128), dt=F16)
            bsum = load("bsum", [128, 4], bsum_t.ap())
            bihn = load("bihn", [128, 2], bihn_t.ap())
            bhhn = load("bhhn", [128, 2], bhhn_t.ap())
            bpmu = load("bpmu", [LAT, 1], bpmu_t.ap())
            bplv = load("bplv", [LAT, 1], bplv_t.ap())
            bae1 = load("bae1", [128, 2], bae1_t.ap())
            bae2 = load("bae2", [128, 1], bae2_t.ap())
            bc1 = load("bc1", [128, 4], bc1_t.ap())
            bc2 = load("bc2", [128, 1], bc2_t.ap())

            onespad = consts.tile([128, 2 * SEG - 1, 1], F16, tag="onespad")
            nc.vector.memset(onespad[:], 0.0)
            nc.vector.memset(onespad[:, SEG - 1, :], 1.0)
            ones64 = consts.tile([LAT, 1], F32, tag="ones64")
            nc.vector.memset(ones64[:], 1.0)
            neg1 = consts.tile([1, 1], F32, tag="neg1")
            nc.vector.memset(neg1[:], -1.0)

            coh = work.tile([128, FREE], F32, tag="coh")
            nc.gpsimd.dma_start(out=coh[:],
                                in_=coh_t.ap().rearrange("(p f) -> p f", p=128))

            # ---- phase 1: per-segment column sums of obs ----
            # lhsT = 8-wide window into a zero-padded ones column; window
            # offset puts the 1.0 column at position s, zeros elsewhere, so
            # every tile accumulates into row s of one (8, 128) PSUM tile.
            sum8_ps = psacc.tile([SEG, OBS], F32, tag="acc")
            for t in range(NT):
                s = t // TPS
                nc.tensor.matmul(sum8_ps[:], rhs=obs_sb[:, t, :],
                                 lhsT=onespad[:, SEG - 1 - s:2 * SEG - 1 - s, :],
                                 start=(t == 0), stop=(t == NT - 1))
            sum8 = work.tile([SEG, OBS], F32, tag="sum8")
            nc.vector.tensor_copy(sum8[:], sum8_ps[:])
            aggT_ps = ps.tile([128, SEG], F32, tag="sm")
            nc.tensor.matmul(aggT_ps[:], lhsT=sum8[:], rhs=ident[0:SEG, 0:SEG],
                             start=True, stop=True)
            aggT = work.tile([128, SEG], F32, tag="aggT")
            nc.vector.tensor_scalar_mul(aggT[:], aggT_ps[:], 1.0 / PTS)

            # ---- GRU over [z_prev, action, coh_scalar] (transposed) ----
            gall_ps = psg.tile([128, 8, SEG], F32, tag="gall")
            g_ps = gall_ps[:, 0:4, :]
            gin_ps = gall_ps[:, 4:6, :]
            ghn_ps = gall_ps[:, 6:8, :]
            for j in range(4):
                nc.tensor.matmul(g_ps[:, j, :],
                                 lhsT=wih[:, j * 128:(j + 1) * 128], rhs=xT[:],
                                 start=True, stop=False)
                for c in range(2):
                    nc.tensor.matmul(g_ps[:, j, :],
                                     lhsT=whh[:, c, j * 128:(j + 1) * 128],
                                     rhs=hpT16[:, c, :], start=False,
                                     stop=(c == 1))
            for j in range(2):
                jj = 4 + j
                nc.tensor.matmul(gin_ps[:, j, :],
                                 lhsT=wih[:, jj * 128:(jj + 1) * 128],
                                 rhs=xT[:], start=True, stop=True)
                for c in range(2):
                    nc.tensor.matmul(ghn_ps[:, j, :],
                                     lhsT=whh[:, c, jj * 128:(jj + 1) * 128],
                                     rhs=hpT16[:, c, :], start=(c == 0),
                                     stop=(c == 1))

            r_sb = work.tile([128, 2, SEG], F32, tag="r")
            u_sb = work.tile([128, 2, SEG], F32, tag="u")
            for j in range(2):
                nc.scalar.activation(r_sb[:, j, :], g_ps[:, j, :], AF.Sigmoid,
                                     bias=bsum[:, j:j + 1])
                nc.scalar.activation(u_sb[:, j, :], g_ps[:, 2 + j, :],
                                     AF.Sigmoid, bias=bsum[:, 2 + j:3 + j])
            hT = work.tile([128, 2, SEG], F32, tag="hT")
            t1 = work.tile([128, 2, SEG], F32, tag="t1")
            t2 = work.tile([128, 2, SEG], F32, tag="t2")
            t3 = work.tile([128, 2, SEG], F32, tag="t3")
            n_sb = work.tile([128, 2, SEG], F32, tag="n")
            omu = work.tile([128, 2, SEG], F32, tag="omu")
            t5 = work.tile([128, 2, SEG], F32, tag="t5")
            t6 = work.tile([128, 2, SEG], F32, tag="t6")
            for j in range(2):
                # n = tanh(gi_n + b_ihn + r * (gh_n + b_hhn))
                nc.scalar.activation(t1[:, j, :], ghn_ps[:, j, :], AF.Identity,
                                     bias=bhhn[:, j:j + 1])
                nc.vector.tensor_mul(t2[:, j, :], r_sb[:, j, :], t1[:, j, :])
                nc.vector.tensor_add(t3[:, j, :], t2[:, j, :], gin_ps[:, j, :])
                nc.scalar.activation(n_sb[:, j, :], t3[:, j, :], AF.Tanh,
                                     bias=bihn[:, j:j + 1])
                # h = (1 - u) * n + u * h_prev
                nc.vector.tensor_scalar(omu[:, j, :], u_sb[:, j, :], -1.0, 1.0,
                                        op0=mybir.AluOpType.mult,
                                        op1=mybir.AluOpType.add)
                nc.vector.tensor_mul(t5[:, j, :], omu[:, j, :], n_sb[:, j, :])
                nc.vector.tensor_mul(t6[:, j, :], u_sb[:, j, :], hpT[:, j, :])
                nc.vector.tensor_add(hT[:, j, :], t5[:, j, :], t6[:, j, :])
            nc.sync.dma_start(out=hT_t.ap().rearrange("(c p) s -> p c s", p=128),
                              in_=hT[:])

            # ---- posterior head: mu_q (=z), logvar_q, uncertainty ----
            muq_ps = ps.tile([LAT, SEG], F32, tag="sm")
            lvq_ps = ps.tile([LAT, SEG], F32, tag="sm")
            post_rhs = [hT[:, 0, :], hT[:, 1, :], aggT[:]]
            for c in range(3):
                nc.tensor.matmul(muq_ps[:], lhsT=wpost[:, c, 0:LAT],
                                 rhs=post_rhs[c], start=(c == 0), stop=(c == 2))
                nc.tensor.matmul(lvq_ps[:], lhsT=wpost[:, c, LAT:2 * LAT],
                                 rhs=post_rhs[c], start=(c == 0), stop=(c == 2))
            zT = work.tile([LAT, SEG], F32, tag="zT")
            lvqT = work.tile([LAT, SEG], F32, tag="lvqT")
            nc.scalar.activation(zT[:], muq_ps[:], AF.Identity, bias=bpmu[:])
            nc.scalar.activation(lvqT[:], lvq_ps[:], AF.Identity, bias=bplv[:])
            nc.sync.dma_start(out=zT_t.ap(), in_=zT[:])

            uncs_ps = ps.tile([SEG, 1], F32, tag="sm")
            nc.tensor.matmul(uncs_ps[:], lhsT=lvqT[:], rhs=ones64[:],
                             start=True, stop=True)
            unc_bias = consts.tile([SEG, 1], F32, tag="unc_bias")
            nc.vector.memset(unc_bias[:], 0.5 * LAT * LOG2PIE)
            unc_sb = work.tile([SEG, 1], F32, tag="unc")
            nc.scalar.activation(unc_sb[:], uncs_ps[:], AF.Identity,
                                 bias=unc_bias[:], scale=0.5)
            nc.sync.dma_start(out=unc_t.ap(), in_=unc_sb[:])

            # ---- per-point priority + segment softmax ----
            # coh layout: partition p holds points [p*256, (p+1)*256);
            # segment of partition p is p // 16.
            unc128_ps = ps.tile([128, 1], F32, tag="sm")
            nc.tensor.matmul(unc128_ps[:], lhsT=rmat[:], rhs=unc_sb[:],
                             start=True, stop=True)
            unc128 = work.tile([128, 1], F32, tag="unc128")
            nc.vector.tensor_copy(unc128[:], unc128_ps[:])

            prio = work.tile([128, FREE], F32, tag="prio")
            nc.vector.tensor_scalar_mul(prio[:], coh[:], unc128[:])
            nc.sync.dma_start(out=prio_t.ap().rearrange("(p f) -> p f", p=128),
                              in_=prio[:])

            pmax = work.tile([128, 1], F32, tag="pmax")
            nc.vector.reduce_max(pmax[:], prio[:], axis=mybir.AxisListType.X)
            pmaxT_ps = ps.tile([1, 128], F32, tag="sm")
            nc.tensor.matmul(pmaxT_ps[:], lhsT=pmax[:], rhs=ident[:],
                             start=True, stop=True)
            mT = work.tile([1, SEG], F32, tag="mT")
            nc.vector.reduce_max(mT[:],
                                 pmaxT_ps[:].rearrange("p (a b) -> p a b", b=16),
                                 axis=mybir.AxisListType.X)
            # negm128[p] = -mT[0, p//16]: expand the repeat on DVE (matmul
            # weight APs allow only one free dim), then one K=1 matmul
            mT128 = work.tile([1, 128], F32, tag="mT128")
            nc.vector.tensor_copy(mT128[:].rearrange("p (a b) -> p a b", b=16),
                                  mT[:].to_broadcast((1, SEG, 16)))
            negm128_ps = ps.tile([128, 1], F32, tag="sm")
            nc.tensor.matmul(negm128_ps[:], lhsT=mT128[:],
                             rhs=neg1[:], start=True, stop=True)
            negm128 = work.tile([128, 1], F32, tag="negm128")
            nc.vector.tensor_copy(negm128[:], negm128_ps[:])

            e_sb = work.tile([128, FREE], F32, tag="e")
            nc.scalar.activation(e_sb[:], prio[:], AF.Exp, bias=negm128[:])
            esum = work.tile([128, 1], F32, tag="esum")
            nc.vector.reduce_sum(esum[:], e_sb[:], axis=mybir.AxisListType.X)
            # den128[p] = sum over p's 16-partition group = GG.T @ esum in one
            # matmul (GG is the block-diagonal ones matrix)
            den128_ps = ps.tile([128, 1], F32, tag="sm")
            nc.tensor.matmul(den128_ps[:], lhsT=ggmat[:], rhs=esum[:],
                             start=True, stop=True)
            rden128 = work.tile([128, 1], F32, tag="rden128")
            nc.vector.reciprocal(rden128[:], den128_ps[:])

            pn_sb = work.tile([128, FREE], F32, tag="pn")
            nc.vector.tensor_scalar_mul(pn_sb[:], e_sb[:], rden128[:])
            nc.sync.dma_start(out=pn_t.ap().rearrange("(p f) -> p f", p=128),
                              in_=pn_sb[:])

            # transpose pn into point-on-partition columns inside a
            # zero-padded zone buffer (zone SEG-1 holds the data):
            # col b*128+q of the zone = pn for points q*256 + b*128 + [0,128)
            pnpad = work.tile([128, 2 * SEG - 1, 2 * OBS], F16, tag="pnpad")
            nc.vector.memset(pnpad[:], 0.0)
            for b in range(2):
                pnT_ps = ps.tile([128, 128], F32, tag="sm")
                nc.tensor.matmul(pnT_ps[:], lhsT=pn_sb[:, b * 128:(b + 1) * 128],
                                 rhs=ident[:], start=True, stop=True)
                nc.vector.tensor_copy(pnpad[:, SEG - 1, b * 128:(b + 1) * 128],
                                      pnT_ps[:])

            # ---- phase C: attended = sum_i pn_i * obs_i per segment ----
            att8_ps = psacc.tile([SEG, OBS], F32, tag="acc")
            for t in range(NT):
                s = t // TPS
                cc = (t % 2) * 128 + t // 2
                nc.tensor.matmul(att8_ps[:], rhs=obs_sb[:, t, :],
                                 lhsT=pnpad[:, SEG - 1 - s:2 * SEG - 1 - s,
                                            cc:cc + 1],
                                 start=(t == 0), stop=(t == NT - 1))
            att8 = work.tile([SEG, OBS], F32, tag="att8")
            nc.vector.tensor_copy(att8[:], att8_ps[:])
            attT_ps2 = ps.tile([128, SEG], F32, tag="sm")
            nc.tensor.matmul(attT_ps2[:], lhsT=att8[:], rhs=ident[0:SEG, 0:SEG],
                             start=True, stop=True)
            attT = work.tile([128, SEG], F16, tag="attT")
            nc.vector.tensor_copy(attT[:], attT_ps2[:])
            hT16 = work.tile([128, 2, SEG], F16, tag="hT16")
            nc.vector.tensor_copy(hT16[:], hT[:])
            zT16 = work.tile([LAT, SEG], F16, tag="zT16")
            nc.vector.tensor_copy(zT16[:], zT[:])

            # ---- obs_enc MLP ----
            ae_ps = psmlp.tile([128, 2, SEG], F32, tag="mlp")
            for j in range(2):
                nc.tensor.matmul(ae_ps[:, j, :],
                                 lhsT=wae1[:, j * 128:(j + 1) * 128],
                                 rhs=attT[:], start=True, stop=True)
            t1ae = work.tile([128, 2, SEG], F16, tag="t1ae")
            for j in range(2):
                nc.scalar.activation(t1ae[:, j, :], ae_ps[:, j, :], AF.Relu,
                                     bias=bae1[:, j:j + 1])
            enc_ps = psmlp.tile([128, 1, SEG], F32, tag="mlp")
            for c in range(2):
                nc.tensor.matmul(enc_ps[:, 0, :], lhsT=wae2[:, c, :],
                                 rhs=t1ae[:, c, :], start=(c == 0),
                                 stop=(c == 1))
            encT = work.tile([128, SEG], F16, tag="encT")
            nc.scalar.activation(encT[:], enc_ps[:, 0, :], AF.Identity,
                                 bias=bae2[:])

            # ---- context MLP ----
            c1_ps = psmlp.tile([128, 4, SEG], F32, tag="mlp")
            c1_rhs = [(wc1_c0, hT16[:, 0, :]), (wc1_c1, hT16[:, 1, :]),
                      (wc1_c2, zT16[:]), (wc1_c3, encT[:])]
            for j in range(4):
                for c, (w, rhs) in enumerate(c1_rhs):
                    nc.tensor.matmul(c1_ps[:, j, :],
                                     lhsT=w[:, j * 128:(j + 1) * 128],
                                     rhs=rhs, start=(c == 0), stop=(c == 3))
            c1_sb = work.tile([128, 4, SEG], F16, tag="c1")
            for j in range(4):
                nc.scalar.activation(c1_sb[:, j, :], c1_ps[:, j, :], AF.Relu,
                                     bias=bc1[:, j:j + 1])
            ctx_ps = psmlp.tile([128, 1, SEG], F32, tag="mlp")
            for c in range(4):
                nc.tensor.matmul(ctx_ps[:, 0, :], lhsT=wc2[:, c, :],
                                 rhs=c1_sb[:, c, :], start=(c == 0),
                                 stop=(c == 3))
            ctxT = work.tile([CTX, SEG], F32, tag="ctxT")
            nc.scalar.activation(ctxT[:], ctx_ps[:, 0, :], AF.Identity,
                                 bias=bc2[:])
            nc.sync.dma_start(out=ctxT_t.ap(), in_=ctxT[:])

    nc.compile()
    return nc


_NC_CACHE = None


def _get_nc():
    global _NC_CACHE
    if _NC_CACHE is None:
        _NC_CACHE = _build_bass()
    return _NC_CACHE


def _host_consts(action, coh_scalar, h_prev, z_prev, W_ih, b_ih, W_hh, b_hh,
                 W_post, b_post, W_ae1, b_ae1, W_ae2, b_ae2, W_c1, b_c1,
                 W_c2, b_c2):
    c = lambda a: np.ascontiguousarray(a, dtype=np.float32)
    x = np.concatenate([z_prev, action, coh_scalar], axis=1)  # (B, 70)
    bsum = (b_ih + b_hh)[:512].reshape(4, 128).T
    rmat = np.zeros((SEG, 128), np.float32)
    rmat[np.arange(128) // 16, np.arange(128)] = 1.0
    consts = {
        "wihT": c(W_ih.T), "whhT": c(W_hh.T).astype(np.float16),
        "wpostT": c(W_post.T),
        "wae1T": c(W_ae1.T).astype(np.float16),
        "wae2T": c(W_ae2.T).astype(np.float16),
        "wc1T": c(W_c1.T).astype(np.float16),
        "wc2T": c(W_c2.T).astype(np.float16),
        "bsumT": c(bsum),
        "bihnT": c(b_ih[512:].reshape(2, 128).T),
        "bhhnT": c(b_hh[512:].reshape(2, 128).T),
        "bpostMu": c(b_post[:LAT, None]), "bpostLv": c(b_post[LAT:, None]),
        "bae1T": c(b_ae1.reshape(2, 128).T), "bae2T": c(b_ae2[:, None]),
        "bc1T": c(b_c1.reshape(4, 128).T), "bc2T": c(b_c2[:, None]),
        "ident": np.eye(128, dtype=np.float32),
        "rmat": rmat,
        "ggmat": (rmat.T @ rmat).astype(np.float32),
    }
    return x, consts


def _reference_numpy(obs, action, coh_scalar, coh_spatial, h_prev, z_prev,
                     batch, W_ih, b_ih, W_hh, b_hh, W_prior, b_prior, W_post,
                     b_post, W_ae1, b_ae1, W_ae2, b_ae2, W_c1, b_c1, W_c2,
                     b_c2):
    """Pure-numpy fallback for a batch layout the device path doesn't cover."""
    def seg_sum(x, idx, nseg):
        out = np.zeros((nseg,) + x.shape[1:], np.float32)
        np.add.at(out, idx, x)
        return out

    nb = batch.astype(np.int64)
    counts = seg_sum(np.ones(len(nb), np.float32), nb, B)
    obs_sum = seg_sum(obs, nb, B)
    obs_agg = obs_sum / np.maximum(counts, 1.0)[:, None]
    x = np.concatenate([z_prev, action, coh_scalar], axis=-1)
    gi = x @ W_ih.T + b_ih
    gh = h_prev @ W_hh.T + b_hh
    gi_r, gi_z, gi_n = np.split(gi, 3, axis=-1)
    gh_r, gh_z, gh_n = np.split(gh, 3, axis=-1)
    sig = lambda v: 1.0 / (1.0 + np.exp(-v))
    r = sig(gi_r + gh_r)
    u = sig(gi_z + gh_z)
    n = np.tanh(gi_n + r * gh_n)
    h = (1.0 - u) * n + u * h_prev
    post = np.concatenate([h, obs_agg], axis=-1) @ W_post.T + b_post
    mu_q, logvar_q = np.split(post, 2, axis=-1)
    z = mu_q
    unc = 0.5 * np.sum(logvar_q + LOG2PIE, axis=-1)
    priority = coh_spatial * unc[nb]
    s = priority
    m = np.full((B,), -np.inf, np.float32)
    np.maximum.at(m, nb, s)
    e = np.exp(s - m[nb])
    denom = seg_sum(e, nb, B)
    pn = e / np.maximum(denom, 1e-12)[nb]
    att = seg_sum(obs * pn[:, None], nb, B)
    enc = np.maximum(att @ W_ae1.T + b_ae1, 0.0) @ W_ae2.T + b_ae2
    ctx_in = np.concatenate([h, z, enc], axis=-1)
    context = np.maximum(ctx_in @ W_c1.T + b_c1, 0.0) @ W_c2.T + b_c2
    return (h.astype(np.float32), z.astype(np.float32),
            context.astype(np.float32), priority.astype(np.float32),
            pn.astype(np.float32), unc.astype(np.float32))


def kernel(**inputs):
    f = {k: np.asarray(v) for k, v in inputs.items()}
    batch = f["batch"]
    expected = np.repeat(np.arange(B, dtype=batch.dtype), PTS)
    if batch.shape != expected.shape or not np.array_equal(batch, expected):
        return _reference_numpy(**{k: (np.asarray(v, np.float32)
                                       if k != "batch" else v)
                                   for k, v in f.items()})

    g = {k: np.ascontiguousarray(np.asarray(v), dtype=np.float32)
         for k, v in f.items() if k != "batch"}
    x, consts = _host_consts(
        g["action"], g["coh_scalar"], g["h_prev"], g["z_prev"],
        g["W_ih"], g["b_ih"], g["W_hh"], g["b_hh"], g["W_post"], g["b_post"],
        g["W_ae1"], g["b_ae1"], g["W_ae2"], g["b_ae2"], g["W_c1"], g["b_c1"],
        g["W_c2"], g["b_c2"])

    obs16 = g["obs"].astype(np.float16)
    in_maps = []
    for d in range(NCORES):
        sl = slice(d * SEG, (d + 1) * SEG)
        psl = slice(d * PPC, (d + 1) * PPC)
        m = dict(consts)
        m["obs"] = np.ascontiguousarray(
            obs16[psl].reshape(NT, 128, OBS).transpose(1, 0, 2)
            .reshape(128, NT * OBS))
        m["coh"] = np.ascontiguousarray(g["coh_spatial"][psl])
        m["xT"] = np.ascontiguousarray(x[sl].T)
        m["hpT"] = np.ascontiguousarray(g["h_prev"][sl].T)
        m["hpT16"] = m["hpT"].astype(np.float16)
        in_maps.append(m)

    nc = _get_nc()
    res = run_bass_kernel_spmd(nc, in_maps, core_ids=list(range(NCORES)))

    h = np.empty((B, HID), np.float32)
    z = np.empty((B, LAT), np.float32)
    context = np.empty((B, CTX), np.float32)
    priority = np.empty((N,), np.float32)
    pn = np.empty((N,), np.float32)
    unc = np.empty((B,), np.float32)
    for d in range(NCORES):
        r = res.results[d]
        sl = slice(d * SEG, (d + 1) * SEG)
        psl = slice(d * PPC, (d + 1) * PPC)
        h[sl] = r["hT"].T
        z[sl] = r["zT"].T
        context[sl] = r["ctxT"].T
        priority[psl] = r["prio"]
        pn[psl] = r["pn"]
        unc[sl] = r["unc"][:, 0]
    return h, z, context, priority, pn, unc


# revision 10
# speedup vs baseline: 1.0935x; 1.0749x over previous
"""Trainium2 Bass kernel for nn_AgentLayerC_v2 (segment_reduce).

Data-parallel over the B=64 segment dim: 8 contiguous segments (32768
points) per NeuronCore. All segment reductions are core-local; the small
GRU/MLP weights are replicated. The obs shard (16.8 MB) is kept
SBUF-resident so HBM reads it exactly once; both heavy reductions
(segment mean, priority-weighted segment sum) run as TensorE
column-reduce matmuls against the resident copy.
"""

import sys

if "/opt/trn_rl_repo" not in sys.path:
    sys.path.insert(0, "/opt/trn_rl_repo")

import numpy as np

import concourse.bass as bass
import concourse.bacc as bacc
import concourse.tile as tile
from concourse import mybir
from concourse.bass_utils import run_bass_kernel_spmd

F32 = mybir.dt.float32
F16 = mybir.dt.float16
AF = mybir.ActivationFunctionType

B = 64
PTS = 4096
N = B * PTS
OBS = 128
ACTD = 5
HID = 256
LAT = 64
CTX = 128
LOG2PIE = float(np.log(2.0 * np.pi * np.e))

NCORES = 8
SEG = B // NCORES          # 8 segments per core
PPC = SEG * PTS            # 32768 points per core
NT = PPC // 128            # 256 point-tiles of 128 per core
TPS = PTS // 128           # 32 tiles per segment
FREE = PPC // 128          # 256 free elems/partition in coh layout
XIN = LAT + ACTD + 1       # 70


def _build_bass():
    nc = bacc.Bacc("TRN2", target_bir_lowering=False, debug=False,
                   num_devices=NCORES)

    di = {}

    def inp(name, shape):
        di[name] = nc.dram_tensor(name, shape, F32, kind="ExternalInput")
        return di[name]

    def outp(name, shape):
        di[name] = nc.dram_tensor(name, shape, F32, kind="ExternalOutput")
        return di[name]

    # host pre-swizzles obs to partition-major: obs_pre[p, t*OBS+c] =
    # obs[t*128+p, c] -> each partition reads one contiguous run per chunk
    obs_t = di.setdefault("obs", nc.dram_tensor("obs", (128, NT * OBS), F16,
                                                 kind="ExternalInput"))
    coh_t = inp("coh", (PPC,))
    xT_t = inp("xT", (XIN, SEG))
    hpT_t = inp("hpT", (HID, SEG))
    wih_t = inp("wihT", (XIN, 3 * HID))
    whh_t = di.setdefault("whhT", nc.dram_tensor("whhT", (HID, 3 * HID), F16,
                                                 kind="ExternalInput"))
    hpT16_t = di.setdefault("hpT16", nc.dram_tensor("hpT16", (HID, SEG), F16,
                                                    kind="ExternalInput"))
    wpost_t = inp("wpostT", (HID + OBS, 2 * LAT))
    wae1_t = di.setdefault("wae1T", nc.dram_tensor("wae1T", (OBS, 256), F16,
                                                    kind="ExternalInput"))
    wae2_t = di.setdefault("wae2T", nc.dram_tensor("wae2T", (256, OBS), F16,
                                                   kind="ExternalInput"))
    wc1_t = di.setdefault("wc1T", nc.dram_tensor("wc1T", (HID + LAT + OBS, 512),
                                                 F16, kind="ExternalInput"))
    wc2_t = di.setdefault("wc2T", nc.dram_tensor("wc2T", (512, CTX), F16,
                                                 kind="ExternalInput"))
    bsum_t = inp("bsumT", (128, 4))
    bihn_t = inp("bihnT", (128, 2))
    bhhn_t = inp("bhhnT", (128, 2))
    bpmu_t = inp("bpostMu", (LAT, 1))
    bplv_t = inp("bpostLv", (LAT, 1))
    bae1_t = inp("bae1T", (128, 2))
    bae2_t = inp("bae2T", (128, 1))
    bc1_t = inp("bc1T", (128, 4))
    bc2_t = inp("bc2T", (128, 1))
    ident_t = inp("ident", (128, 128))
    rmat_t = inp("rmat", (SEG, 128))
    ggmat_t = inp("ggmat", (128, 128))

    prio_t = outp("prio", (PPC,))
    pn_t = outp("pn", (PPC,))
    hT_t = outp("hT", (HID, SEG))
    zT_t = outp("zT", (LAT, SEG))
    ctxT_t = outp("ctxT", (CTX, SEG))
    unc_t = outp("unc", (SEG, 1))

    with tile.TileContext(nc) as tc:
        with (
            tc.tile_pool(name="consts", bufs=1) as consts,
            tc.tile_pool(name="obsp", bufs=1) as obsp,
            tc.tile_pool(name="work", bufs=1) as work,
            tc.tile_pool(name="ps", bufs=3, space="PSUM") as ps,
            tc.tile_pool(name="psacc", bufs=1, space="PSUM") as psacc,
            tc.tile_pool(name="psg", bufs=1, space="PSUM") as psg,
            tc.tile_pool(name="psmlp", bufs=2, space="PSUM") as psmlp,
        ):
            # ---- obs resident load first: it owns the HWDGE queues ----
            obs_sb = obsp.tile([128, NT, OBS], F16, tag="obs")
            NCHUNK = 16
            CW = NT * OBS // NCHUNK
            TPC = NT // NCHUNK
            for g in range(NCHUNK):
                eng = nc.sync if g % 2 == 0 else nc.scalar
                eng.dma_start(out=obs_sb[:, g * TPC:(g + 1) * TPC, :],
                              in_=obs_t.ap()[:, g * CW:(g + 1) * CW])

            # ---- constants / weights into SBUF (gpsimd queue) ----
            def load(name, shape, src_ap, dt=F32):
                t = consts.tile(shape, dt, tag=name)
                nc.gpsimd.dma_start(out=t[:], in_=src_ap)
                return t

            ident = load("ident", [128, 128], ident_t.ap())
            rmat = load("rmat", [SEG, 128], rmat_t.ap())
            ggmat = load("ggmat", [128, 128], ggmat_t.ap())
            xT = load("xT", [XIN, SEG], xT_t.ap())
            hpT = load("hpT", [128, 2, SEG],
                       hpT_t.ap().rearrange("(c p) s -> p c s", p=128))
            wih = load("wih", [XIN, 3 * HID], wih_t.ap())
            whh = load("whh", [128, 2, 3 * HID],
                       whh_t.ap().rearrange("(c p) n -> p c n", p=128), dt=F16)
            hpT16 = load("hpT16", [128, 2, SEG],
                         hpT16_t.ap().rearrange("(c p) s -> p c s", p=128),
                         dt=F16)
            wpost = load("wpost", [128, 3, 2 * LAT],
                         wpost_t.ap().rearrange("(c p) n -> p c n", p=128))
            wae1 = load("wae1", [OBS, 256], wae1_t.ap(), dt=F16)
            wae2 = load("wae2", [128, 2, OBS],
                        wae2_t.ap().rearrange("(c p) n -> p c n", p=128), dt=F16)
            wc1_c0 = load("wc1_c0", [128, 512], wc1_t.ap()[0:128, :], dt=F16)
            wc1_c1 = load("wc1_c1", [128, 512], wc1_t.ap()[128:256, :], dt=F16)
            wc1_c2 = load("wc1_c2", [LAT, 512], wc1_t.ap()[256:320, :], dt=F16)
            wc1_c3 = load("wc1_c3", [128, 512], wc1_t.ap()[320:448, :], dt=F16)
            wc2 = load("wc2", [128, 4, CTX],
                       wc2_t.ap().rearrange("(c p) n -> p c n", p=128), dt=F16)
            bsum = load("bsum", [128, 4], bsum_t.ap())
            bihn = load("bihn", [128, 2], bihn_t.ap())
            bhhn = load("bhhn", [128, 2], bhhn_t.ap())
            bpmu = load("bpmu", [LAT, 1], bpmu_t.ap())
            bplv = load("bplv", [LAT, 1], bplv_t.ap())
            bae1 = load("bae1", [128, 2], bae1_t.ap())
            bae2 = load("bae2", [128, 1], bae2_t.ap())
            bc1 = load("bc1", [128, 4], bc1_t.ap())
            bc2 = load("bc2", [128, 1], bc2_t.ap())

            onespad = consts.tile([128, 2 * SEG - 1, 1], F16, tag="onespad")
            nc.vector.memset(onespad[:], 0.0)
            nc.vector.memset(onespad[:, SEG - 1, :], 1.0)
            ones64 = consts.tile([LAT, 1], F32, tag="ones64")
            nc.vector.memset(ones64[:], 1.0)
            neg1 = consts.tile([1, 1], F32, tag="neg1")
            nc.vector.memset(neg1[:], -1.0)

            coh = work.tile([128, FREE], F32, tag="coh")
            nc.gpsimd.dma_start(out=coh[:],
                                in_=coh_t.ap().rearrange("(p f) -> p f", p=128))

            # ---- phase 1: per-segment column sums of obs ----
            # lhsT = 8-wide window into a zero-padded ones column; window
            # offset puts the 1.0 column at position s, zeros elsewhere, so
            # every tile accumulates into row s of one (8, 128) PSUM tile.
            sum8_ps = psacc.tile([SEG, OBS], F32, tag="acc")
            for t in range(NT):
                s = t // TPS
                nc.tensor.matmul(sum8_ps[:], rhs=obs_sb[:, t, :],
                                 lhsT=onespad[:, SEG - 1 - s:2 * SEG - 1 - s, :],
                                 start=(t == 0), stop=(t == NT - 1))
            sum8 = work.tile([SEG, OBS], F32, tag="sum8")
            nc.vector.tensor_copy(sum8[:], sum8_ps[:])
            aggT_ps = ps.tile([128, SEG], F32, tag="sm")
            nc.tensor.matmul(aggT_ps[:], lhsT=sum8[:], rhs=ident[0:SEG, 0:SEG],
                             start=True, stop=True)
            aggT = work.tile([128, SEG], F32, tag="aggT")
            nc.vector.tensor_scalar_mul(aggT[:], aggT_ps[:], 1.0 / PTS)

            # ---- GRU over [z_prev, action, coh_scalar] (transposed) ----
            gall_ps = psg.tile([128, 8, SEG], F32, tag="gall")
            g_ps = gall_ps[:, 0:4, :]
            gin_ps = gall_ps[:, 4:6, :]
            ghn_ps = gall_ps[:, 6:8, :]
            for j in range(4):
                nc.tensor.matmul(g_ps[:, j, :],
                                 lhsT=wih[:, j * 128:(j + 1) * 128], rhs=xT[:],
                                 start=True, stop=False)
                for c in range(2):
                    nc.tensor.matmul(g_ps[:, j, :],
                                     lhsT=whh[:, c, j * 128:(j + 1) * 128],
                                     rhs=hpT16[:, c, :], start=False,
                                     stop=(c == 1))
            for j in range(2):
                jj = 4 + j
                nc.tensor.matmul(gin_ps[:, j, :],
                                 lhsT=wih[:, jj * 128:(jj + 1) * 128],
                                 rhs=xT[:], start=True, stop=True)
                for c in range(2):
                    nc.tensor.matmul(ghn_ps[:, j, :],
                                     lhsT=whh[:, c, jj * 128:(jj + 1) * 128],
                                     rhs=hpT16[:, c, :], start=(c == 0),
                                     stop=(c == 1))

            r_sb = work.tile([128, 2, SEG], F32, tag="r")
            u_sb = work.tile([128, 2, SEG], F32, tag="u")
            for j in range(2):
                nc.scalar.activation(r_sb[:, j, :], g_ps[:, j, :], AF.Sigmoid,
                                     bias=bsum[:, j:j + 1])
                nc.scalar.activation(u_sb[:, j, :], g_ps[:, 2 + j, :],
                                     AF.Sigmoid, bias=bsum[:, 2 + j:3 + j])
            hT = work.tile([128, 2, SEG], F32, tag="hT")
            t1 = work.tile([128, 2, SEG], F32, tag="t1")
            t2 = work.tile([128, 2, SEG], F32, tag="t2")
            t3 = work.tile([128, 2, SEG], F32, tag="t3")
            n_sb = work.tile([128, 2, SEG], F32, tag="n")
            omu = work.tile([128, 2, SEG], F32, tag="omu")
            t5 = work.tile([128, 2, SEG], F32, tag="t5")
            t6 = work.tile([128, 2, SEG], F32, tag="t6")
            for j in range(2):
                # n = tanh(gi_n + b_ihn + r * (gh_n + b_hhn))
                nc.scalar.activation(t1[:, j, :], ghn_ps[:, j, :], AF.Identity,
                                     bias=bhhn[:, j:j + 1])
                nc.vector.tensor_mul(t2[:, j, :], r_sb[:, j, :], t1[:, j, :])
                nc.vector.tensor_add(t3[:, j, :], t2[:, j, :], gin_ps[:, j, :])
                nc.scalar.activation(n_sb[:, j, :], t3[:, j, :], AF.Tanh,
                                     bias=bihn[:, j:j + 1])
                # h = (1 - u) * n + u * h_prev
                nc.vector.tensor_scalar(omu[:, j, :], u_sb[:, j, :], -1.0, 1.0,
                                        op0=mybir.AluOpType.mult,
                                        op1=mybir.AluOpType.add)
                nc.vector.tensor_mul(t5[:, j, :], omu[:, j, :], n_sb[:, j, :])
                nc.vector.tensor_mul(t6[:, j, :], u_sb[:, j, :], hpT[:, j, :])
                nc.vector.tensor_add(hT[:, j, :], t5[:, j, :], t6[:, j, :])
            nc.sync.dma_start(out=hT_t.ap().rearrange("(c p) s -> p c s", p=128),
                              in_=hT[:])

            # ---- posterior head: mu_q (=z), logvar_q, uncertainty ----
            muq_ps = ps.tile([LAT, SEG], F32, tag="sm")
            lvq_ps = ps.tile([LAT, SEG], F32, tag="sm")
            post_rhs = [hT[:, 0, :], hT[:, 1, :], aggT[:]]
            for c in range(3):
                nc.tensor.matmul(muq_ps[:], lhsT=wpost[:, c, 0:LAT],
                                 rhs=post_rhs[c], start=(c == 0), stop=(c == 2))
                nc.tensor.matmul(lvq_ps[:], lhsT=wpost[:, c, LAT:2 * LAT],
                                 rhs=post_rhs[c], start=(c == 0), stop=(c == 2))
            zT = work.tile([LAT, SEG], F32, tag="zT")
            lvqT = work.tile([LAT, SEG], F32, tag="lvqT")
            nc.scalar.activation(zT[:], muq_ps[:], AF.Identity, bias=bpmu[:])
            nc.scalar.activation(lvqT[:], lvq_ps[:], AF.Identity, bias=bplv[:])
            nc.sync.dma_start(out=zT_t.ap(), in_=zT[:])

            uncs_ps = ps.tile([SEG, 1], F32, tag="sm")
            nc.tensor.matmul(uncs_ps[:], lhsT=lvqT[:], rhs=ones64[:],
                             start=True, stop=True)
            unc_bias = consts.tile([SEG, 1], F32, tag="unc_bias")
            nc.vector.memset(unc_bias[:], 0.5 * LAT * LOG2PIE)
            unc_sb = work.tile([SEG, 1], F32, tag="unc")
            nc.scalar.activation(unc_sb[:], uncs_ps[:], AF.Identity,
                                 bias=unc_bias[:], scale=0.5)
            nc.sync.dma_start(out=unc_t.ap(), in_=unc_sb[:])

            # ---- per-point priority + segment softmax ----
            # coh layout: partition p holds points [p*256, (p+1)*256);
            # segment of partition p is p // 16.
            unc128_ps = ps.tile([128, 1], F32, tag="sm")
            nc.tensor.matmul(unc128_ps[:], lhsT=rmat[:], rhs=unc_sb[:],
                             start=True, stop=True)
            unc128 = work.tile([128, 1], F32, tag="unc128")
            nc.vector.tensor_copy(unc128[:], unc128_ps[:])

            prio = work.tile([128, FREE], F32, tag="prio")
            nc.vector.tensor_scalar_mul(prio[:], coh[:], unc128[:])
            nc.sync.dma_start(out=prio_t.ap().rearrange("(p f) -> p f", p=128),
                              in_=prio[:])

            pmax = work.tile([128, 1], F32, tag="pmax")
            nc.vector.reduce_max(pmax[:], prio[:], axis=mybir.AxisListType.X)
            pmaxT_ps = ps.tile([1, 128], F32, tag="sm")
            nc.tensor.matmul(pmaxT_ps[:], lhsT=pmax[:], rhs=ident[:],
                             start=True, stop=True)
            mT = work.tile([1, SEG], F32, tag="mT")
            nc.vector.reduce_max(mT[:],
                                 pmaxT_ps[:].rearrange("p (a b) -> p a b", b=16),
                                 axis=mybir.AxisListType.X)
            # negm128[p] = -mT[0, p//16]: expand the repeat on DVE (matmul
            # weight APs allow only one free dim), then one K=1 matmul
            mT128 = work.tile([1, 128], F32, tag="mT128")
            nc.vector.tensor_copy(mT128[:].rearrange("p (a b) -> p a b", b=16),
                                  mT[:].to_broadcast((1, SEG, 16)))
            negm128_ps = ps.tile([128, 1], F32, tag="sm")
            nc.tensor.matmul(negm128_ps[:], lhsT=mT128[:],
                             rhs=neg1[:], start=True, stop=True)
            negm128 = work.tile([128, 1], F32, tag="negm128")
            nc.vector.tensor_copy(negm128[:], negm128_ps[:])

            e_sb = work.tile([128, FREE], F32, tag="e")
            nc.scalar.activation(e_sb[:], prio[:], AF.Exp, bias=negm128[:])
            esum = work.tile([128, 1], F32, tag="esum")
            nc.vector.reduce_sum(esum[:], e_sb[:], axis=mybir.AxisListType.X)
            # den128[p] = sum over p's 16-partition group = GG.T @ esum in one
            # matmul (GG is the block-diagonal ones matrix)
            den128_ps = ps.tile([128, 1], F32, tag="sm")
            nc.tensor.matmul(den128_ps[:], lhsT=ggmat[:], rhs=esum[:],
                             start=True, stop=True)
            rden128 = work.tile([128, 1], F32, tag="rden128")
            nc.vector.reciprocal(rden128[:], den128_ps[:])

            pn_sb = work.tile([128, FREE], F32, tag="pn")
            nc.vector.tensor_scalar_mul(pn_sb[:], e_sb[:], rden128[:])
            nc.sync.dma_start(out=pn_t.ap().rearrange("(p f) -> p f", p=128),
                              in_=pn_sb[:])

            # transpose pn into point-on-partition columns inside a
            # zero-padded zone buffer (zone SEG-1 holds the data):
            # col b*128+q of the zone = pn for points q*256 + b*128 + [0,128)
            pnpad = work.tile([128, 2 * SEG - 1, 2 * OBS], F16, tag="pnpad")
            nc.vector.memset(pnpad[:], 0.0)
            for b in range(2):
                pnT_ps = ps.tile([128, 128], F32, tag="sm")
                nc.tensor.matmul(pnT_ps[:], lhsT=pn_sb[:, b * 128:(b + 1) * 128],
                                 rhs=ident[:], start=True, stop=True)
                nc.vector.tensor_copy(pnpad[:, SEG - 1, b * 128:(b + 1) * 128],
                                      pnT_ps[:])

            # ---- phase C: attended = sum_i pn_i * obs_i per segment ----
            att8_ps = psacc.tile([SEG, OBS], F32, tag="acc")
            for t in range(NT):
                s = t // TPS
                cc = (t % 2) * 128 + t // 2
                nc.tensor.matmul(att8_ps[:], rhs=obs_sb[:, t, :],
                                 lhsT=pnpad[:, SEG - 1 - s:2 * SEG - 1 - s,
                                            cc:cc + 1],
                                 start=(t == 0), stop=(t == NT - 1))
            att8 = work.tile([SEG, OBS], F32, tag="att8")
            nc.vector.tensor_copy(att8[:], att8_ps[:])
            attT_ps2 = ps.tile([128, SEG], F32, tag="sm")
            nc.tensor.matmul(attT_ps2[:], lhsT=att8[:], rhs=ident[0:SEG, 0:SEG],
                             start=True, stop=True)
            attT = work.tile([128, SEG], F16, tag="attT")
            nc.vector.tensor_copy(attT[:], attT_ps2[:])
            hT16 = work.tile([128, 2, SEG], F16, tag="hT16")
            nc.vector.tensor_copy(hT16[:], hT[:])
            zT16 = work.tile([LAT, SEG], F16, tag="zT16")
            nc.vector.tensor_copy(zT16[:], zT[:])

            # ---- obs_enc MLP ----
            ae_ps = psmlp.tile([128, 2, SEG], F32, tag="mlp")
            for j in range(2):
                nc.tensor.matmul(ae_ps[:, j, :],
                                 lhsT=wae1[:, j * 128:(j + 1) * 128],
                                 rhs=attT[:], start=True, stop=True)
            t1ae = work.tile([128, 2, SEG], F16, tag="t1ae")
            for j in range(2):
                nc.scalar.activation(t1ae[:, j, :], ae_ps[:, j, :], AF.Relu,
                                     bias=bae1[:, j:j + 1])
            enc_ps = psmlp.tile([128, 1, SEG], F32, tag="mlp")
            for c in range(2):
                nc.tensor.matmul(enc_ps[:, 0, :], lhsT=wae2[:, c, :],
                                 rhs=t1ae[:, c, :], start=(c == 0),
                                 stop=(c == 1))
            encT = work.tile([128, SEG], F16, tag="encT")
            nc.scalar.activation(encT[:], enc_ps[:, 0, :], AF.Identity,
                                 bias=bae2[:])

            # ---- context MLP ----
            c1_ps = psmlp.tile([128, 4, SEG], F32, tag="mlp")
            c1_rhs = [(wc1_c0, hT16[:, 0, :]), (wc1_c1, hT16[:, 1, :]),
                      (wc1_c2, zT16[:]), (wc1_c3, encT[:])]
            for j in range(4):
                for c, (w, rhs) in enumerate(c1_rhs):
                    nc.tensor.matmul(c1_ps[:, j, :],
                                     lhsT=w[:, j * 128:(j + 1) * 128],
                                     rhs=rhs, start=(c == 0), stop=(c == 3))
            c1_sb = work.tile([128, 4, SEG], F16, tag="c1")
            for j in range(4):
                nc.scalar.activation(c1_sb[:, j, :], c1_ps[:, j, :], AF.Relu,
                                     bias=bc1[:, j:j + 1])
            ctx_ps = psmlp.tile([128, 1, SEG], F32, tag="mlp")
            for c in range(4):
                nc.tensor.matmul(ctx_ps[:, 0, :], lhsT=wc2[:, c, :],
                                 rhs=c1_sb[:, c, :], start=(c == 0),
                                 stop=(c == 3))
            ctxT = work.tile([CTX, SEG], F32, tag="ctxT")
            nc.scalar.activation(ctxT[:], ctx_ps[:, 0, :], AF.Identity,
                                 bias=bc2[:])
            nc.sync.dma_start(out=ctxT_t.ap(), in_=ctxT[:])

    nc.compile()
    return nc


_NC_CACHE = None


def _get_nc():
    global _NC_CACHE
    if _NC_CACHE is None:
        _NC_CACHE = _build_bass()
    return _NC_CACHE


def _host_consts(action, coh_scalar, h_prev, z_prev, W_ih, b_ih, W_hh, b_hh,
                 W_post, b_post, W_ae1, b_ae1, W_ae2, b_ae2, W_c1, b_c1,
                 W_c2, b_c2):
    c = lambda a: np.ascontiguousarray(a, dtype=np.float32)
    x = np.concatenate([z_prev, action, coh_scalar], axis=1)  # (B, 70)
    bsum = (b_ih + b_hh)[:512].reshape(4, 128).T
    rmat = np.zeros((SEG, 128), np.float32)
    rmat[np.arange(128) // 16, np.arange(128)] = 1.0
    consts = {
        "wihT": c(W_ih.T), "whhT": c(W_hh.T).astype(np.float16),
        "wpostT": c(W_post.T),
        "wae1T": c(W_ae1.T).astype(np.float16),
        "wae2T": c(W_ae2.T).astype(np.float16),
        "wc1T": c(W_c1.T).astype(np.float16),
        "wc2T": c(W_c2.T).astype(np.float16),
        "bsumT": c(bsum),
        "bihnT": c(b_ih[512:].reshape(2, 128).T),
        "bhhnT": c(b_hh[512:].reshape(2, 128).T),
        "bpostMu": c(b_post[:LAT, None]), "bpostLv": c(b_post[LAT:, None]),
        "bae1T": c(b_ae1.reshape(2, 128).T), "bae2T": c(b_ae2[:, None]),
        "bc1T": c(b_c1.reshape(4, 128).T), "bc2T": c(b_c2[:, None]),
        "ident": np.eye(128, dtype=np.float32),
        "rmat": rmat,
        "ggmat": (rmat.T @ rmat).astype(np.float32),
    }
    return x, consts


def _reference_numpy(obs, action, coh_scalar, coh_spatial, h_prev, z_prev,
                     batch, W_ih, b_ih, W_hh, b_hh, W_prior, b_prior, W_post,
                     b_post, W_ae1, b_ae1, W_ae2, b_ae2, W_c1, b_c1, W_c2,
                     b_c2):
    """Pure-numpy fallback for a batch layout the device path doesn't cover."""
    def seg_sum(x, idx, nseg):
        out = np.zeros((nseg,) + x.shape[1:], np.float32)
        np.add.at(out, idx, x)
        return out

    nb = batch.astype(np.int64)
    counts = seg_sum(np.ones(len(nb), np.float32), nb, B)
    obs_sum = seg_sum(obs, nb, B)
    obs_agg = obs_sum / np.maximum(counts, 1.0)[:, None]
    x = np.concatenate([z_prev, action, coh_scalar], axis=-1)
    gi = x @ W_ih.T + b_ih
    gh = h_prev @ W_hh.T + b_hh
    gi_r, gi_z, gi_n = np.split(gi, 3, axis=-1)
    gh_r, gh_z, gh_n = np.split(gh, 3, axis=-1)
    sig = lambda v: 1.0 / (1.0 + np.exp(-v))
    r = sig(gi_r + gh_r)
    u = sig(gi_z + gh_z)
    n = np.tanh(gi_n + r * gh_n)
    h = (1.0 - u) * n + u * h_prev
    post = np.concatenate([h, obs_agg], axis=-1) @ W_post.T + b_post
    mu_q, logvar_q = np.split(post, 2, axis=-1)
    z = mu_q
    unc = 0.5 * np.sum(logvar_q + LOG2PIE, axis=-1)
    priority = coh_spatial * unc[nb]
    s = priority
    m = np.full((B,), -np.inf, np.float32)
    np.maximum.at(m, nb, s)
    e = np.exp(s - m[nb])
    denom = seg_sum(e, nb, B)
    pn = e / np.maximum(denom, 1e-12)[nb]
    att = seg_sum(obs * pn[:, None], nb, B)
    enc = np.maximum(att @ W_ae1.T + b_ae1, 0.0) @ W_ae2.T + b_ae2
    ctx_in = np.concatenate([h, z, enc], axis=-1)
    context = np.maximum(ctx_in @ W_c1.T + b_c1, 0.0) @ W_c2.T + b_c2
    return (h.astype(np.float32), z.astype(np.float32),
            context.astype(np.float32), priority.astype(np.float32),
            pn.astype(np.float32), unc.astype(np.float32))


def kernel(**inputs):
    f = {k: np.asarray(v) for k, v in inputs.items()}
    batch = f["batch"]
    expected = np.repeat(np.arange(B, dtype=batch.dtype), PTS)
    if batch.shape != expected.shape or not np.array_equal(batch, expected):
        return _reference_numpy(**{k: (np.asarray(v, np.float32)
                                       if k != "batch" else v)
                                   for k, v in f.items()})

    g = {k: np.ascontiguousarray(np.asarray(v), dtype=np.float32)
         for k, v in f.items() if k != "batch"}
    x, consts = _host_consts(
        g["action"], g["coh_scalar"], g["h_prev"], g["z_prev"],
        g["W_ih"], g["b_ih"], g["W_hh"], g["b_hh"], g["W_post"], g["b_post"],
        g["W_ae1"], g["b_ae1"], g["W_ae2"], g["b_ae2"], g["W_c1"], g["b_c1"],
        g["W_c2"], g["b_c2"])

    obs16 = g["obs"].astype(np.float16)
    in_maps = []
    for d in range(NCORES):
        sl = slice(d * SEG, (d + 1) * SEG)
        psl = slice(d * PPC, (d + 1) * PPC)
        m = dict(consts)
        m["obs"] = np.ascontiguousarray(
            obs16[psl].reshape(NT, 128, OBS).transpose(1, 0, 2)
            .reshape(128, NT * OBS))
        m["coh"] = np.ascontiguousarray(g["coh_spatial"][psl])
        m["xT"] = np.ascontiguousarray(x[sl].T)
        m["hpT"] = np.ascontiguousarray(g["h_prev"][sl].T)
        m["hpT16"] = m["hpT"].astype(np.float16)
        in_maps.append(m)

    nc = _get_nc()
    res = run_bass_kernel_spmd(nc, in_maps, core_ids=list(range(NCORES)))

    h = np.empty((B, HID), np.float32)
    z = np.empty((B, LAT), np.float32)
    context = np.empty((B, CTX), np.float32)
    priority = np.empty((N,), np.float32)
    pn = np.empty((N,), np.float32)
    unc = np.empty((B,), np.float32)
    for d in range(NCORES):
        r = res.results[d]
        sl = slice(d * SEG, (d + 1) * SEG)
        psl = slice(d * PPC, (d + 1) * PPC)
        h[sl] = r["hT"].T
        z[sl] = r["zT"].T
        context[sl] = r["ctxT"].T
        priority[psl] = r["prio"]
        pn[psl] = r["pn"]
        unc[sl] = r["unc"][:, 0]
    return h, z, context, priority, pn, unc
